# revision 7
# baseline (speedup 1.0000x reference)
"""Cross-attention (1x1-conv q/k/v + softmax(Q^T K) + V@attn^T) on Trainium2.

Data-parallel over batch: 8 batches -> 8 NeuronCores, one full [N,N]
attention per core; the small CxC projection weights are replicated.

Per-core device program (all matmuls, zero transposes). The two score
projections are folded into one on the host: scores = (Wq x1)^T (Wk x2)
= x1^T G x2 with G = Wk^T Wq [CxC], so x1 feeds the score matmuls raw:
  A[c,m]   = G.T @ x2              (fp16 result, c on partitions)
  vT[m,c'] = x2.T @ WvT            (fp32r matmul, bf16 result; appended
                                    ones column c'=C)
  sT[m,n]  = A.T @ x1              (fp16 stationary x fp32r moving scores,
                                    transposed layout)
  pT[m,n]  = exp(sT - SHIFT)       (ScalarE, bf16 out; SHIFT makes per-row max
                                    subtraction unnecessary: softmax is
                                    shift-invariant and scores stay in
                                    [-150, ~110] => exp in fp32/bf16 range)
  o'[n,c'] = pT.T @ vT             (bf16; ones column accumulates row sums)
  outT[n,c] = o'[n,:C] * (1/o'[n,C])

dtype choices: the moving operands stay fp32r (1 cycle/row for free dims >=
256, and input rounding is amplified sqrt(C)x by the projections so x1/x2
need the precision). The STATIONARY operands of the score path (G, A) are
fp16: LDWEIGHTS for a 2-byte stationary takes ~95ns vs 187ns for fp32r, and
the 187ns load stream was the score-phase bottleneck (227ns/matmul observed
vs the 213ns matmul roofline). fp16's 10-bit mantissa adds only ~2e-3 abs
score error (scores have std ~16), far inside the error budget; bf16 would
not be (8x coarser). The value path (pT, vT) is bf16: pT needs bf16's
exponent range (unnormalized exp up to e^50) and vT feeds 257-row matmuls
whose bf16 LDWEIGHTS (~95-111ns) roughly hides under the 107ns matmul.

Prologue: the runtime starts executing program instructions at ~7.2us; each
dma_start costs ~670-800ns of serial DIRECT2D descriptor processing on its
trigger engine. Both SP (sync) and Activation (scalar) have HW DGE queues,
so the first transfers are split across both: scalar triggers the first x2
half-quarters + x1 head while sync triggers the weights, halving the
time-to-first-byte. x DMAs are 512-col for the first two chunks (so the
first k projection starts after wk + 0.5MB instead of wk + 1MB) and
priority-chained (the SDMA engines round-robin across queued transfers, so
without ordering every DMA finishes together and the PE idles).

The host reassembles outT -> [B, C, H, W].

Biases are not applied: the problem spec fixes bq/bk/bv to zeros.
"""

from contextlib import ExitStack

import numpy as np

import concourse.bass as bass
import concourse.mybir as mybir
import concourse.tile as tile
from concourse import bacc, bass_utils

B, C, H, W = 8, 256, 64, 64
N = H * W          # 4096 tokens per image
P = 128            # partition count
KC = C // P        # 2 contraction chunks over channels
NMM = N // P       # 32 key-side chunks
SB = 512           # query-side superblock (score matmul free dim)
NSB = N // SB      # 8
C2 = C + 1         # value width + ones column (bf16 matmuls allow odd free)
SHIFT = 60.0       # softmax exp shift (see module docstring)

_CACHE: dict = {}
TRACE = False       # set by test harness to capture an NTFF profile
TRACE_DIR = None    # optional fixed profile output dir


def _build_program():
    f32 = mybir.dt.float32
    f32r = mybir.dt.float32r   # moving operands: full-rate PE, ~TF32 precision
    f16 = mybir.dt.float16     # score-path stationaries: fast LDWEIGHTS
    bf16 = mybir.dt.bfloat16   # value path: exp range + fast LDWEIGHTS
    exp = mybir.ActivationFunctionType.Exp
    # bacc (not raw Bass): its compile() pass splits multi-semaphore waits,
    # which walrus codegen requires (one wait per TPB instruction).
    nc = bacc.Bacc("TRN2", target_bir_lowering=False, debug=False)

    x1_d = nc.dram_tensor("x1", [C, N], f16, kind="ExternalInput").ap()
    x2_d = nc.dram_tensor("x2", [C, N], f16, kind="ExternalInput").ap()
    wk_d = nc.dram_tensor("wkT", [C, C], f16, kind="ExternalInput").ap()
    wv_d = nc.dram_tensor("wvT", [C, C], f16, kind="ExternalInput").ap()
    outT_d = nc.dram_tensor("outT", [N, C], f16, kind="ExternalOutput").ap()

    def r(ap):  # DRAM-side view matching the fp32r tile dtype (bit-identical)
        return ap.bitcast(f32r)

    with tile.TileContext(nc) as tc:
        with ExitStack() as ctx:
            consts = ctx.enter_context(tc.tile_pool(name="consts", bufs=1))
            acts = ctx.enter_context(tc.tile_pool(name="acts", bufs=1))
            xpool = ctx.enter_context(tc.tile_pool(name="xpool", bufs=1))

            # ---- input DMAs first: triggers cost ~670-800ns of serial
            # descriptor processing per dma_start, and nothing else in the
            # program may delay them. The first three x chunks go on the
            # scalar (Activation) HW-DGE queue, in parallel with sync
            # triggering the weights.
            wk_sb = consts.tile([P, KC, C], f16, name="wk_sb")
            wv_sb = consts.tile([P, KC, C], f16, name="wv_sb")
            # x chunk tables: (tile, start_n, width). First two x2/x1 chunks
            # are 512 wide so the first projections start after 0.5MB.
            QT = N // 4
            x2_chunks = []
            x1_chunks = []
            for nm, tbl, widths in (("x2", x2_chunks, (QT, QT, QT, QT)),
                                    ("x1", x1_chunks, (QT, QT, QT, QT))):
                n0 = 0
                for i, wd in enumerate(widths):
                    tbl.append((xpool.tile([P, KC, wd], f16,
                                           name=f"{nm}_{i}"), n0, wd))
                    n0 += wd
            x2_r = x2_d.rearrange("(kc p) n -> p kc n", p=P)
            x1_r = x1_d.rearrange("(kc p) n -> p kc n", p=P)

            def chain(eng, specs, prev=None):
                for dst, src, n0, wd in specs:
                    dma = eng.dma_start(out=dst, in_=src[:, :, n0:n0 + wd])
                    if prev is not None:
                        tile.add_dep_helper(dma.ins, prev.ins,
                                            reason="dma priority chain")
                    prev = dma
                return prev

            # Two concurrent priority chains, one per HW-DGE trigger engine:
            # scalar streams the x2 quarters while sync streams wk, wv, then
            # the x1 quarters. Within a chain transfers are serialized (the
            # SDMA engines round-robin across queued transfers, so an
            # unordered queue finishes everything at once and the PE idles);
            # across the two chains the 2x concurrency roughly doubles
            # aggregate bandwidth, so x1q0 lands at ~12us instead of ~16.5.
            chain(nc.scalar, [
                (x2_chunks[0][0], x2_r, 0, QT),
                (x2_chunks[1][0], x2_r, 1024, QT),
                (x2_chunks[2][0], x2_r, 2048, QT),
                (x2_chunks[3][0], x2_r, 3072, QT),
            ])
            wkdma = nc.sync.dma_start(
                out=wk_sb, in_=wk_d.rearrange("(kc p) c -> p kc c", p=P))
            wvdma = nc.sync.dma_start(
                out=wv_sb, in_=wv_d.rearrange("(kc p) c -> p kc c", p=P))
            tile.add_dep_helper(wvdma.ins, wkdma.ins, reason="dma priority chain")
            chain(nc.sync, [
                (x1_chunks[0][0], x1_r, 0, QT),
                (x1_chunks[1][0], x1_r, 1024, QT),
                (x1_chunks[2][0], x1_r, 2048, QT),
                (x1_chunks[3][0], x1_r, 3072, QT),
            ], prev=wvdma)

            def xs(tbl, n0, wd):
                # slice [n0, n0+wd) out of the chunk table (never straddles)
                for t, start, width in tbl:
                    if start <= n0 and n0 + wd <= start + width:
                        return t[:, :, n0 - start:n0 - start + wd]
                raise AssertionError((n0, wd))

            def xs_kc(tbl, kc, n0, wd):
                for t, start, width in tbl:
                    if start <= n0 and n0 + wd <= start + width:
                        return t[:, kc, n0 - start:n0 - start + wd]
                raise AssertionError((n0, wd))

            nbias = consts.tile([P, 1], f32)
            nc.vector.memset(nbias, -SHIFT)

            warm = consts.tile([P, SB], f16, name="warm")
            nc.vector.memset(warm, 0.0)

            # A (folded k-side) per-superblock tiles in fp16, vT per m-chunk:
            # fine-grained deps let scores/out matmuls start before all
            # projections finish.
            k_sb = [acts.tile([P, KC, SB], f16, name=f"k_{ns}", bufs=1)
                    for ns in range(NSB)]
            vT_sb = [acts.tile([P, C2], bf16, name=f"vT_{mm}", bufs=1)
                     for mm in range(NMM)]
            for mm in range(NMM):
                nc.vector.memset(vT_sb[mm][:, C:C2], 1.0)

            # ---- pools (ps/po PSUM rotations are shared by projections
            # and the attention loop; 6 + 2 = all 8 banks) ----
            pts = ctx.enter_context(tc.tile_pool(name="pts", bufs=24))
            ps_pool = ctx.enter_context(tc.tile_pool(name="ps", bufs=3, space="PSUM"))
            po_pool = ctx.enter_context(tc.tile_pool(name="po", bufs=2, space="PSUM"))
            outp = ctx.enter_context(tc.tile_pool(name="outp", bufs=4))
            normp = ctx.enter_context(tc.tile_pool(name="normp", bufs=4))

            def emit_kqproj(ns):
                # one [P,2,SB] psum tile per n-chunk; kc-outer so consecutive
                # matmuls alternate PSUM banks
                pq = ps_pool.tile([P, 2, SB], f32, tag="ps", name=f"pq_{ns}")
                for kc in range(KC):
                    for mo in range(KC):
                        nc.tensor.matmul(
                            pq[:, mo, :],
                            lhsT=wk_sb[:, kc, mo * P:(mo + 1) * P],
                            rhs=xs_kc(x2_chunks, kc, ns * SB, SB),
                            start=(kc == 0), stop=(kc == KC - 1))
                for mo in range(KC):
                    nc.vector.tensor_copy(out=k_sb[ns][:, mo, :],
                                          in_=pq[:, mo, :])

            def emit_vproj(mm0, count):
                # m-chunks [mm0, mm0+count) of the value projection; pairs
                # of accumulators from the po rotation alternate banks
                for pr in range(count // 2):
                    pv = [po_pool.tile([P, C], f32, tag="po",
                                       name=f"pv_{mm0}_{pr}_{i}")
                          for i in range(2)]
                    for kc in range(KC):
                        for i in range(2):
                            mm = mm0 + pr * 2 + i
                            nc.tensor.matmul(
                                pv[i],
                                lhsT=xs_kc(x2_chunks, kc, mm * P, P),
                                rhs=wv_sb[:, kc, :],
                                start=(kc == 0), stop=(kc == KC - 1))
                    for i in range(2):
                        nc.vector.tensor_copy(
                            out=vT_sb[mm0 + pr * 2 + i][:, 0:C],
                            in_=pv[i])

            def emit_scores(sb, t, pt_tiles):
                ps = ps_pool.tile([P, 2, SB], f32, tag="ps",
                                  name=f"ps_{sb}_{t}")
                for kc in range(KC):   # kc-outer: banks alternate A B A B
                    for i in range(2):
                        koff = (t * 2 + i) * P
                        kt = k_sb[koff // SB]
                        nc.tensor.matmul(
                            ps[:, i, :],
                            lhsT=kt[:, kc, koff % SB:koff % SB + P],
                            rhs=xs_kc(x1_chunks, kc, sb * SB, SB),
                            start=(kc == 0), stop=(kc == KC - 1))
                pt = pts.tile([P, 2, SB], bf16, tag="pt")
                nc.scalar.activation(out=pt, in_=ps, func=exp,
                                     bias=nbias, scale=1.0)
                pt_tiles.append(pt)

            def emit_out(sb, pt_tiles):
                # j-outer: one live out-accumulator bank at a time.
                for j in range(SB // P):
                    po = po_pool.tile([P, C2], f32, tag="po",
                                      name=f"po_{sb}_{j}")
                    for mm in range(NMM):
                        nc.tensor.matmul(
                            po,
                            lhsT=pt_tiles[mm // 2][:, mm % 2,
                                                   j * P:(j + 1) * P],
                            rhs=vT_sb[mm],
                            start=(mm == 0), stop=(mm == NMM - 1))
                    rc = normp.tile([P, 1], f32, tag="rc")
                    nc.vector.reciprocal(rc, po[:, C:C + 1])
                    ot = outp.tile([P, C], f16, tag="ot")
                    nc.vector.tensor_scalar_mul(ot, po[:, 0:C], rc)
                    n0 = sb * SB + j * P
                    nc.sync.dma_start(out=outT_d[n0:n0 + P, :], in_=ot)

            # ---- PE warm-up: dummy matmuls from program start until x2q0
            # lands keep the PE continuously busy so the ~3.5us p-state ramp
            # completes before the first real matmul (which then runs at
            # 2.4GHz instead of 1.2). Sized to end just before x2q0. ----
            wps = ps_pool.tile([P, 2, SB], f32, tag="ps", name="warm_ps")
            for wi in range(5):
                nc.tensor.matmul(wps[:, wi % 2, :], lhsT=warm[:, 0:P],
                                 rhs=warm, start=True, stop=True)

            # ---- prologue: k/v projections hand-interleaved with the first
            # superblock's scores, following the DMA arrival order, so the PE
            # never drains while x2/x1 chunks trickle in ----
            pt0 = []
            for qt in range(4):
                emit_kqproj(qt * 2)
                emit_kqproj(qt * 2 + 1)
                emit_vproj(qt * 8, 8)
                for t in range(qt * 4, qt * 4 + 4):
                    emit_scores(0, t, pt0)
            emit_out(0, pt0)

            for sb in range(1, NSB):
                pt_tiles = []
                for t in range(NMM // 2):
                    emit_scores(sb, t, pt_tiles)
                emit_out(sb, pt_tiles)
    nc.compile()
    return nc


def _get_program():
    if "nc" not in _CACHE:
        _CACHE["nc"] = _build_program()
    return _CACHE["nc"]


def kernel(**inputs) -> np.ndarray:
    x1 = np.ascontiguousarray(np.asarray(inputs["x1"], np.float16)).reshape(B, C, N)
    x2 = np.ascontiguousarray(np.asarray(inputs["x2"], np.float16)).reshape(B, C, N)
    # scores = (Wq x1)^T (Wk x2) = x1^T (Wq^T Wk) x2: fold both score
    # projections into one by shipping G = Wk^T Wq as the k-side weight;
    # x1 then feeds the score matmuls raw (saves 32 matmuls/core and one
    # fp32r rounding on the q side).
    G = (np.asarray(inputs["Wk"], np.float64).T
         @ np.asarray(inputs["Wq"], np.float64))
    wkT = np.ascontiguousarray(G.astype(np.float16))
    wvT = np.ascontiguousarray(np.asarray(inputs["Wv"], np.float16).T)

    in_maps = [
        {"x1": x1[b], "x2": x2[b], "wkT": wkT, "wvT": wvT}
        for b in range(B)
    ]
    nc = _get_program()
    res = bass_utils.run_bass_kernel_spmd(nc, in_maps, core_ids=list(range(B)),
                                          trace=TRACE, tmpdir=TRACE_DIR)
    _CACHE["last_results"] = res
    out = np.empty((B, C, N), np.float32)
    for b in range(B):
        out[b] = res.results[b]["outT"].T.astype(np.float32)
    return out.reshape(B, C, H, W)


if __name__ == "__main__":
    nc = _build_program()
    n = sum(len(b.instructions) for b in nc.m.functions[0].blocks)
    print(f"program built ok: {n} instructions")


# revision 8
# speedup vs baseline: 1.0261x; 1.0261x over previous
"""Cross-attention (1x1-conv q/k/v + softmax(Q^T K) + V@attn^T) on Trainium2.

Data-parallel over batch: 8 batches -> 8 NeuronCores, one full [N,N]
attention per core; the small CxC projection weights are replicated.

Per-core device program (all matmuls, zero transposes). The two score
projections are folded into one on the host: scores = (Wq x1)^T (Wk x2)
= x1^T G x2 with G = Wk^T Wq [CxC], so x1 feeds the score matmuls raw:
  A[c,m]   = G.T @ x2              (fp16 result, c on partitions)
  vT[m,c'] = x2.T @ WvT            (fp32r matmul, bf16 result; appended
                                    ones column c'=C)
  sT[m,n]  = A.T @ x1              (fp16 stationary x fp32r moving scores,
                                    transposed layout)
  pT[m,n]  = exp(sT - SHIFT)       (ScalarE, bf16 out; SHIFT makes per-row max
                                    subtraction unnecessary: softmax is
                                    shift-invariant and scores stay in
                                    [-150, ~110] => exp in fp32/bf16 range)
  o'[n,c'] = pT.T @ vT             (bf16; ones column accumulates row sums)
  outT[n,c] = o'[n,:C] * (1/o'[n,C])

dtype choices: the moving operands stay fp32r (1 cycle/row for free dims >=
256, and input rounding is amplified sqrt(C)x by the projections so x1/x2
need the precision). The STATIONARY operands of the score path (G, A) are
fp16: LDWEIGHTS for a 2-byte stationary takes ~95ns vs 187ns for fp32r, and
the 187ns load stream was the score-phase bottleneck (227ns/matmul observed
vs the 213ns matmul roofline). fp16's 10-bit mantissa adds only ~2e-3 abs
score error (scores have std ~16), far inside the error budget; bf16 would
not be (8x coarser). The value path (pT, vT) is bf16: pT needs bf16's
exponent range (unnormalized exp up to e^50) and vT feeds 257-row matmuls
whose bf16 LDWEIGHTS (~95-111ns) roughly hides under the 107ns matmul.

Prologue: the runtime starts executing program instructions at ~7.2us; each
dma_start costs ~670-800ns of serial DIRECT2D descriptor processing on its
trigger engine. Both SP (sync) and Activation (scalar) have HW DGE queues,
so the first transfers are split across both: scalar triggers the first x2
half-quarters + x1 head while sync triggers the weights, halving the
time-to-first-byte. x DMAs are 512-col for the first two chunks (so the
first k projection starts after wk + 0.5MB instead of wk + 1MB) and
priority-chained (the SDMA engines round-robin across queued transfers, so
without ordering every DMA finishes together and the PE idles).

The host reassembles outT -> [B, C, H, W].

Biases are not applied: the problem spec fixes bq/bk/bv to zeros.
"""

from contextlib import ExitStack

import numpy as np

import concourse.bass as bass
import concourse.mybir as mybir
import concourse.tile as tile
from concourse import bacc, bass_utils

B, C, H, W = 8, 256, 64, 64
N = H * W          # 4096 tokens per image
P = 128            # partition count
KC = C // P        # 2 contraction chunks over channels
NMM = N // P       # 32 key-side chunks
SB = 512           # query-side superblock (score matmul free dim)
NSB = N // SB      # 8
C2 = C + 1         # value width + ones column (bf16 matmuls allow odd free)
SHIFT = 60.0       # softmax exp shift (see module docstring)

_CACHE: dict = {}
TRACE = False       # set by test harness to capture an NTFF profile
TRACE_DIR = None    # optional fixed profile output dir


def _build_program():
    f32 = mybir.dt.float32
    f32r = mybir.dt.float32r   # moving operands: full-rate PE, ~TF32 precision
    f16 = mybir.dt.float16     # score-path stationaries: fast LDWEIGHTS
    bf16 = mybir.dt.bfloat16   # value path: exp range + fast LDWEIGHTS
    exp = mybir.ActivationFunctionType.Exp
    # bacc (not raw Bass): its compile() pass splits multi-semaphore waits,
    # which walrus codegen requires (one wait per TPB instruction).
    nc = bacc.Bacc("TRN2", target_bir_lowering=False, debug=False)

    x1_d = nc.dram_tensor("x1", [C, N], f16, kind="ExternalInput").ap()
    x2_d = nc.dram_tensor("x2", [C, N], f16, kind="ExternalInput").ap()
    wk_d = nc.dram_tensor("wkT", [C, C], f16, kind="ExternalInput").ap()
    wv_d = nc.dram_tensor("wvT", [C, C], f16, kind="ExternalInput").ap()
    outT_d = nc.dram_tensor("outT", [N, C], f16, kind="ExternalOutput").ap()

    def r(ap):  # DRAM-side view matching the fp32r tile dtype (bit-identical)
        return ap.bitcast(f32r)

    with tile.TileContext(nc) as tc:
        with ExitStack() as ctx:
            consts = ctx.enter_context(tc.tile_pool(name="consts", bufs=1))
            acts = ctx.enter_context(tc.tile_pool(name="acts", bufs=1))
            xpool = ctx.enter_context(tc.tile_pool(name="xpool", bufs=1))

            # ---- input DMAs first: triggers cost ~670-800ns of serial
            # descriptor processing per dma_start, and nothing else in the
            # program may delay them. The first three x chunks go on the
            # scalar (Activation) HW-DGE queue, in parallel with sync
            # triggering the weights.
            wk_sb = consts.tile([P, KC, C], f16, name="wk_sb")
            wv_sb = consts.tile([P, KC, C], f16, name="wv_sb")
            # x chunk tables: (tile, start_n, width). First two x2/x1 chunks
            # are 512 wide so the first projections start after 0.5MB.
            QT = N // 4
            x2_chunks = []
            x1_chunks = []
            for nm, tbl, widths in (("x2", x2_chunks, (QT, QT, QT, QT)),
                                    ("x1", x1_chunks, (QT, QT, QT, QT))):
                n0 = 0
                for i, wd in enumerate(widths):
                    tbl.append((xpool.tile([P, KC, wd], f16,
                                           name=f"{nm}_{i}"), n0, wd))
                    n0 += wd
            x2_r = x2_d.rearrange("(kc p) n -> p kc n", p=P)
            x1_r = x1_d.rearrange("(kc p) n -> p kc n", p=P)

            def chain(eng, specs, prev=None):
                for dst, src, n0, wd in specs:
                    dma = eng.dma_start(out=dst, in_=src[:, :, n0:n0 + wd])
                    if prev is not None:
                        tile.add_dep_helper(dma.ins, prev.ins,
                                            reason="dma priority chain")
                    prev = dma
                return prev

            # scalar: first x2 quarter + first x1 quarter (chained).
            # sync: the small weights in parallel (wk gates the first
            # matmul), then the remaining x chunks chained behind the scalar
            # head. DMA bandwidth in this phase is a shared pool, so chains
            # are serialized by priority rather than run concurrently.
            sc_tail = chain(nc.scalar, [
                (x2_chunks[0][0], x2_r, 0, QT),
                (x1_chunks[0][0], x1_r, 0, QT),
            ])
            wkdma = nc.sync.dma_start(
                out=wk_sb, in_=wk_d.rearrange("(kc p) c -> p kc c", p=P))
            wvdma = nc.sync.dma_start(
                out=wv_sb, in_=wv_d.rearrange("(kc p) c -> p kc c", p=P))
            tile.add_dep_helper(wvdma.ins, wkdma.ins, reason="dma priority chain")
            chain(nc.sync, [
                (x2_chunks[1][0], x2_r, 1024, QT),
                (x2_chunks[2][0], x2_r, 2048, QT),
                (x2_chunks[3][0], x2_r, 3072, QT),
                (x1_chunks[1][0], x1_r, 1024, QT),
                (x1_chunks[2][0], x1_r, 2048, QT),
                (x1_chunks[3][0], x1_r, 3072, QT),
            ], prev=sc_tail)

            def xs(tbl, n0, wd):
                # slice [n0, n0+wd) out of the chunk table (never straddles)
                for t, start, width in tbl:
                    if start <= n0 and n0 + wd <= start + width:
                        return t[:, :, n0 - start:n0 - start + wd]
                raise AssertionError((n0, wd))

            def xs_kc(tbl, kc, n0, wd):
                for t, start, width in tbl:
                    if start <= n0 and n0 + wd <= start + width:
                        return t[:, kc, n0 - start:n0 - start + wd]
                raise AssertionError((n0, wd))

            nbias = consts.tile([P, 1], f32)
            nc.vector.memset(nbias, -SHIFT)

            warm = consts.tile([P, SB], f16, name="warm")
            nc.vector.memset(warm, 0.0)

            # A (folded k-side) per-superblock tiles in fp16, vT per m-chunk:
            # fine-grained deps let scores/out matmuls start before all
            # projections finish.
            k_sb = [acts.tile([P, KC, SB], f16, name=f"k_{ns}", bufs=1)
                    for ns in range(NSB)]
            vT_sb = [acts.tile([P, C2], bf16, name=f"vT_{mm}", bufs=1)
                     for mm in range(NMM)]
            for mm in range(NMM):
                nc.vector.memset(vT_sb[mm][:, C:C2], 1.0)

            # ---- pools (ps/po PSUM rotations are shared by projections
            # and the attention loop; 6 + 2 = all 8 banks) ----
            pts = ctx.enter_context(tc.tile_pool(name="pts", bufs=24))
            ps_pool = ctx.enter_context(tc.tile_pool(name="ps", bufs=3, space="PSUM"))
            po_pool = ctx.enter_context(tc.tile_pool(name="po", bufs=2, space="PSUM"))
            outp = ctx.enter_context(tc.tile_pool(name="outp", bufs=4))
            normp = ctx.enter_context(tc.tile_pool(name="normp", bufs=4))

            def emit_kqproj(ns):
                # one [P,2,SB] psum tile per n-chunk; kc-outer so consecutive
                # matmuls alternate PSUM banks
                pq = ps_pool.tile([P, 2, SB], f32, tag="ps", name=f"pq_{ns}")
                for kc in range(KC):
                    for mo in range(KC):
                        nc.tensor.matmul(
                            pq[:, mo, :],
                            lhsT=wk_sb[:, kc, mo * P:(mo + 1) * P],
                            rhs=xs_kc(x2_chunks, kc, ns * SB, SB),
                            start=(kc == 0), stop=(kc == KC - 1))
                for mo in range(KC):
                    nc.vector.tensor_copy(out=k_sb[ns][:, mo, :],
                                          in_=pq[:, mo, :])

            def emit_vproj(mm0, count):
                # m-chunks [mm0, mm0+count) of the value projection; pairs
                # of accumulators from the po rotation alternate banks
                for pr in range(count // 2):
                    pv = [po_pool.tile([P, C], f32, tag="po",
                                       name=f"pv_{mm0}_{pr}_{i}")
                          for i in range(2)]
                    for kc in range(KC):
                        for i in range(2):
                            mm = mm0 + pr * 2 + i
                            nc.tensor.matmul(
                                pv[i],
                                lhsT=xs_kc(x2_chunks, kc, mm * P, P),
                                rhs=wv_sb[:, kc, :],
                                start=(kc == 0), stop=(kc == KC - 1))
                    for i in range(2):
                        nc.vector.tensor_copy(
                            out=vT_sb[mm0 + pr * 2 + i][:, 0:C],
                            in_=pv[i])

            def emit_scores(sb, t, pt_tiles):
                ps = ps_pool.tile([P, 2, SB], f32, tag="ps",
                                  name=f"ps_{sb}_{t}")
                for kc in range(KC):   # kc-outer: banks alternate A B A B
                    for i in range(2):
                        koff = (t * 2 + i) * P
                        kt = k_sb[koff // SB]
                        nc.tensor.matmul(
                            ps[:, i, :],
                            lhsT=kt[:, kc, koff % SB:koff % SB + P],
                            rhs=xs_kc(x1_chunks, kc, sb * SB, SB),
                            start=(kc == 0), stop=(kc == KC - 1))
                pt = pts.tile([P, 2, SB], bf16, tag="pt")
                nc.scalar.activation(out=pt, in_=ps, func=exp,
                                     bias=nbias, scale=1.0)
                pt_tiles.append(pt)

            def emit_out(sb, pt_tiles):
                # j-outer: one live out-accumulator bank at a time.
                for j in range(SB // P):
                    po = po_pool.tile([P, C2], f32, tag="po",
                                      name=f"po_{sb}_{j}")
                    for mm in range(NMM):
                        nc.tensor.matmul(
                            po,
                            lhsT=pt_tiles[mm // 2][:, mm % 2,
                                                   j * P:(j + 1) * P],
                            rhs=vT_sb[mm],
                            start=(mm == 0), stop=(mm == NMM - 1))
                    rc = normp.tile([P, 1], f32, tag="rc")
                    nc.vector.reciprocal(rc, po[:, C:C + 1])
                    ot = outp.tile([P, C], f16, tag="ot")
                    nc.vector.tensor_scalar_mul(ot, po[:, 0:C], rc)
                    n0 = sb * SB + j * P
                    nc.sync.dma_start(out=outT_d[n0:n0 + P, :], in_=ot)

            # ---- PE warm-up: dummy matmuls from program start until x2q0
            # lands keep the PE continuously busy so the ~3.5us p-state ramp
            # completes before the first real matmul (which then runs at
            # 2.4GHz instead of 1.2). Sized to end just before x2q0. ----
            wps = ps_pool.tile([P, 2, SB], f32, tag="ps", name="warm_ps")
            for wi in range(5):
                nc.tensor.matmul(wps[:, wi % 2, :], lhsT=warm[:, 0:P],
                                 rhs=warm, start=True, stop=True)

            # ---- prologue: k/v projections hand-interleaved with the first
            # superblock's scores, following the DMA arrival order, so the PE
            # never drains while x2/x1 chunks trickle in ----
            pt0 = []
            for qt in range(4):
                emit_kqproj(qt * 2)
                emit_kqproj(qt * 2 + 1)
                emit_vproj(qt * 8, 8)
                for t in range(qt * 4, qt * 4 + 4):
                    emit_scores(0, t, pt0)
            emit_out(0, pt0)

            for sb in range(1, NSB):
                pt_tiles = []
                for t in range(NMM // 2):
                    emit_scores(sb, t, pt_tiles)
                emit_out(sb, pt_tiles)
    nc.compile()
    return nc


def _get_program():
    if "nc" not in _CACHE:
        _CACHE["nc"] = _build_program()
    return _CACHE["nc"]


def kernel(**inputs) -> np.ndarray:
    x1 = np.ascontiguousarray(np.asarray(inputs["x1"], np.float16)).reshape(B, C, N)
    x2 = np.ascontiguousarray(np.asarray(inputs["x2"], np.float16)).reshape(B, C, N)
    # scores = (Wq x1)^T (Wk x2) = x1^T (Wq^T Wk) x2: fold both score
    # projections into one by shipping G = Wk^T Wq as the k-side weight;
    # x1 then feeds the score matmuls raw (saves 32 matmuls/core and one
    # fp32r rounding on the q side).
    G = (np.asarray(inputs["Wk"], np.float64).T
         @ np.asarray(inputs["Wq"], np.float64))
    wkT = np.ascontiguousarray(G.astype(np.float16))
    wvT = np.ascontiguousarray(np.asarray(inputs["Wv"], np.float16).T)

    in_maps = [
        {"x1": x1[b], "x2": x2[b], "wkT": wkT, "wvT": wvT}
        for b in range(B)
    ]
    nc = _get_program()
    res = bass_utils.run_bass_kernel_spmd(nc, in_maps, core_ids=list(range(B)),
                                          trace=TRACE, tmpdir=TRACE_DIR)
    _CACHE["last_results"] = res
    out = np.empty((B, C, N), np.float32)
    for b in range(B):
        out[b] = res.results[b]["outT"].T.astype(np.float32)
    return out.reshape(B, C, H, W)


if __name__ == "__main__":
    nc = _build_program()
    n = sum(len(b.instructions) for b in nc.m.functions[0].blocks)
    print(f"program built ok: {n} instructions")


# revision 9
# speedup vs baseline: 1.0288x; 1.0026x over previous
"""Cross-attention (1x1-conv q/k/v + softmax(Q^T K) + V@attn^T) on Trainium2.

Data-parallel over batch: 8 batches -> 8 NeuronCores, one full [N,N]
attention per core; the small CxC projection weights are replicated.

Per-core device program (all matmuls, zero transposes). The two score
projections are folded into one on the host: scores = (Wq x1)^T (Wk x2)
= x1^T G x2 with G = Wk^T Wq [CxC], so x1 feeds the score matmuls raw:
  A[c,m]   = G.T @ x2              (fp16 result, c on partitions)
  vT[m,c'] = x2.T @ WvT            (fp32r matmul, bf16 result; appended
                                    ones column c'=C)
  sT[m,n]  = A.T @ x1              (fp16 stationary x fp32r moving scores,
                                    transposed layout)
  pT[m,n]  = exp(sT - SHIFT)       (ScalarE, bf16 out; SHIFT makes per-row max
                                    subtraction unnecessary: softmax is
                                    shift-invariant and scores stay in
                                    [-150, ~110] => exp in fp32/bf16 range)
  o'[n,c'] = pT.T @ vT             (bf16; ones column accumulates row sums)
  outT[n,c] = o'[n,:C] * (1/o'[n,C])

dtype choices: the moving operands stay fp32r (1 cycle/row for free dims >=
256, and input rounding is amplified sqrt(C)x by the projections so x1/x2
need the precision). The STATIONARY operands of the score path (G, A) are
fp16: LDWEIGHTS for a 2-byte stationary takes ~95ns vs 187ns for fp32r, and
the 187ns load stream was the score-phase bottleneck (227ns/matmul observed
vs the 213ns matmul roofline). fp16's 10-bit mantissa adds only ~2e-3 abs
score error (scores have std ~16), far inside the error budget; bf16 would
not be (8x coarser). The value path (pT, vT) is bf16: pT needs bf16's
exponent range (unnormalized exp up to e^50) and vT feeds 257-row matmuls
whose bf16 LDWEIGHTS (~95-111ns) roughly hides under the 107ns matmul.

Prologue: the runtime starts executing program instructions at ~7.2us; each
dma_start costs ~670-800ns of serial DIRECT2D descriptor processing on its
trigger engine. Both SP (sync) and Activation (scalar) have HW DGE queues,
so the first transfers are split across both: scalar triggers the first x2
half-quarters + x1 head while sync triggers the weights, halving the
time-to-first-byte. x DMAs are 512-col for the first two chunks (so the
first k projection starts after wk + 0.5MB instead of wk + 1MB) and
priority-chained (the SDMA engines round-robin across queued transfers, so
without ordering every DMA finishes together and the PE idles).

The host reassembles outT -> [B, C, H, W].

Biases are not applied: the problem spec fixes bq/bk/bv to zeros.
"""

from contextlib import ExitStack

import numpy as np

import concourse.bass as bass
import concourse.mybir as mybir
import concourse.tile as tile
from concourse import bacc, bass_utils

B, C, H, W = 8, 256, 64, 64
N = H * W          # 4096 tokens per image
P = 128            # partition count
KC = C // P        # 2 contraction chunks over channels
NMM = N // P       # 32 key-side chunks
SB = 512           # query-side superblock (score matmul free dim)
NSB = N // SB      # 8
C2 = C + 1         # value width + ones column (bf16 matmuls allow odd free)
SHIFT = 60.0       # softmax exp shift (see module docstring)

_CACHE: dict = {}
TRACE = False       # set by test harness to capture an NTFF profile
TRACE_DIR = None    # optional fixed profile output dir


def _build_program():
    f32 = mybir.dt.float32
    f32r = mybir.dt.float32r   # moving operands: full-rate PE, ~TF32 precision
    f16 = mybir.dt.float16     # score-path stationaries: fast LDWEIGHTS
    bf16 = mybir.dt.bfloat16   # value path: exp range + fast LDWEIGHTS
    exp = mybir.ActivationFunctionType.Exp
    # bacc (not raw Bass): its compile() pass splits multi-semaphore waits,
    # which walrus codegen requires (one wait per TPB instruction).
    nc = bacc.Bacc("TRN2", target_bir_lowering=False, debug=False)

    # x1/x2 arrive pre-arranged by the host as [quarter, partition, kc,
    # n-in-quarter]: each (quarter, partition) is one contiguous 4KB run, so
    # a quarter DMA needs only 128 descriptors of 4KB (vs 256 of 2KB from
    # the natural [C, N] layout) -- the head transfers are descriptor-bound.
    x1_d = nc.dram_tensor("x1", [4, P, KC, N // 4], f16, kind="ExternalInput").ap()
    x2_d = nc.dram_tensor("x2", [4, P, KC, N // 4], f16, kind="ExternalInput").ap()
    wk_d = nc.dram_tensor("wkT", [C, C], f16, kind="ExternalInput").ap()
    wv_d = nc.dram_tensor("wvT", [C, C], f16, kind="ExternalInput").ap()
    outT_d = nc.dram_tensor("outT", [N, C], f16, kind="ExternalOutput").ap()

    def r(ap):  # DRAM-side view matching the fp32r tile dtype (bit-identical)
        return ap.bitcast(f32r)

    with tile.TileContext(nc) as tc:
        with ExitStack() as ctx:
            consts = ctx.enter_context(tc.tile_pool(name="consts", bufs=1))
            acts = ctx.enter_context(tc.tile_pool(name="acts", bufs=1))
            xpool = ctx.enter_context(tc.tile_pool(name="xpool", bufs=1))

            # ---- input DMAs first: triggers cost ~670-800ns of serial
            # descriptor processing per dma_start, and nothing else in the
            # program may delay them. The first three x chunks go on the
            # scalar (Activation) HW-DGE queue, in parallel with sync
            # triggering the weights.
            wk_sb = consts.tile([P, KC, C], f16, name="wk_sb")
            wv_sb = consts.tile([P, KC, C], f16, name="wv_sb")
            # x chunk tables: (tile, start_n, width). First two x2/x1 chunks
            # are 512 wide so the first projections start after 0.5MB.
            QT = N // 4
            x2_chunks = []
            x1_chunks = []
            for nm, tbl, widths in (("x2", x2_chunks, (QT, QT, QT, QT)),
                                    ("x1", x1_chunks, (QT, QT, QT, QT))):
                n0 = 0
                for i, wd in enumerate(widths):
                    tbl.append((xpool.tile([P, KC, wd], f16,
                                           name=f"{nm}_{i}"), n0, wd))
                    n0 += wd
            def chain(eng, specs, prev=None):
                for dst, src, qt in specs:
                    dma = eng.dma_start(out=dst, in_=src[qt])
                    if prev is not None:
                        tile.add_dep_helper(dma.ins, prev.ins,
                                            reason="dma priority chain")
                    prev = dma
                return prev

            # scalar: first x2 quarter + first x1 quarter (chained).
            # sync: the small weights in parallel (wk gates the first
            # matmul), then the remaining x chunks chained behind the scalar
            # head. DMA bandwidth in this phase is a shared pool, so chains
            # are serialized by priority rather than run concurrently.
            sc_tail = chain(nc.scalar, [
                (x2_chunks[0][0], x2_d, 0),
                (x1_chunks[0][0], x1_d, 0),
            ])
            wkdma = nc.sync.dma_start(
                out=wk_sb, in_=wk_d.rearrange("(kc p) c -> p kc c", p=P))
            wvdma = nc.sync.dma_start(
                out=wv_sb, in_=wv_d.rearrange("(kc p) c -> p kc c", p=P))
            tile.add_dep_helper(wvdma.ins, wkdma.ins, reason="dma priority chain")
            chain(nc.sync, [
                (x2_chunks[1][0], x2_d, 1),
                (x2_chunks[2][0], x2_d, 2),
                (x2_chunks[3][0], x2_d, 3),
                (x1_chunks[1][0], x1_d, 1),
                (x1_chunks[2][0], x1_d, 2),
                (x1_chunks[3][0], x1_d, 3),
            ], prev=sc_tail)

            def xs(tbl, n0, wd):
                # slice [n0, n0+wd) out of the chunk table (never straddles)
                for t, start, width in tbl:
                    if start <= n0 and n0 + wd <= start + width:
                        return t[:, :, n0 - start:n0 - start + wd]
                raise AssertionError((n0, wd))

            def xs_kc(tbl, kc, n0, wd):
                for t, start, width in tbl:
                    if start <= n0 and n0 + wd <= start + width:
                        return t[:, kc, n0 - start:n0 - start + wd]
                raise AssertionError((n0, wd))

            nbias = consts.tile([P, 1], f32)
            nc.vector.memset(nbias, -SHIFT)


            # A (folded k-side) per-superblock tiles in fp16, vT per m-chunk:
            # fine-grained deps let scores/out matmuls start before all
            # projections finish.
            k_sb = [acts.tile([P, KC, SB], f16, name=f"k_{ns}", bufs=1)
                    for ns in range(NSB)]
            vT_sb = [acts.tile([P, C2], bf16, name=f"vT_{mm}", bufs=1)
                     for mm in range(NMM)]
            for mm in range(NMM):
                nc.vector.memset(vT_sb[mm][:, C:C2], 1.0)

            # ---- pools (ps/po PSUM rotations are shared by projections
            # and the attention loop; 6 + 2 = all 8 banks) ----
            pts = ctx.enter_context(tc.tile_pool(name="pts", bufs=24))
            ps_pool = ctx.enter_context(tc.tile_pool(name="ps", bufs=3, space="PSUM"))
            po_pool = ctx.enter_context(tc.tile_pool(name="po", bufs=2, space="PSUM"))
            outp = ctx.enter_context(tc.tile_pool(name="outp", bufs=4))
            normp = ctx.enter_context(tc.tile_pool(name="normp", bufs=4))

            def emit_kqproj(ns):
                # one [P,2,SB] psum tile per n-chunk; kc-outer so consecutive
                # matmuls alternate PSUM banks
                pq = ps_pool.tile([P, 2, SB], f32, tag="ps", name=f"pq_{ns}")
                for kc in range(KC):
                    for mo in range(KC):
                        nc.tensor.matmul(
                            pq[:, mo, :],
                            lhsT=wk_sb[:, kc, mo * P:(mo + 1) * P],
                            rhs=xs_kc(x2_chunks, kc, ns * SB, SB),
                            start=(kc == 0), stop=(kc == KC - 1))
                for mo in range(KC):
                    nc.vector.tensor_copy(out=k_sb[ns][:, mo, :],
                                          in_=pq[:, mo, :])

            def emit_vproj(mm0, count):
                # m-chunks [mm0, mm0+count) of the value projection; pairs
                # of accumulators from the po rotation alternate banks
                for pr in range(count // 2):
                    pv = [po_pool.tile([P, C], f32, tag="po",
                                       name=f"pv_{mm0}_{pr}_{i}")
                          for i in range(2)]
                    for kc in range(KC):
                        for i in range(2):
                            mm = mm0 + pr * 2 + i
                            nc.tensor.matmul(
                                pv[i],
                                lhsT=xs_kc(x2_chunks, kc, mm * P, P),
                                rhs=wv_sb[:, kc, :],
                                start=(kc == 0), stop=(kc == KC - 1))
                    for i in range(2):
                        nc.vector.tensor_copy(
                            out=vT_sb[mm0 + pr * 2 + i][:, 0:C],
                            in_=pv[i])

            def emit_scores(sb, t, pt_tiles):
                ps = ps_pool.tile([P, 2, SB], f32, tag="ps",
                                  name=f"ps_{sb}_{t}")
                for kc in range(KC):   # kc-outer: banks alternate A B A B
                    for i in range(2):
                        koff = (t * 2 + i) * P
                        kt = k_sb[koff // SB]
                        nc.tensor.matmul(
                            ps[:, i, :],
                            lhsT=kt[:, kc, koff % SB:koff % SB + P],
                            rhs=xs_kc(x1_chunks, kc, sb * SB, SB),
                            start=(kc == 0), stop=(kc == KC - 1))
                pt = pts.tile([P, 2, SB], bf16, tag="pt")
                nc.scalar.activation(out=pt, in_=ps, func=exp,
                                     bias=nbias, scale=1.0)
                pt_tiles.append(pt)

            def emit_out(sb, pt_tiles):
                # j-outer: one live out-accumulator bank at a time.
                for j in range(SB // P):
                    po = po_pool.tile([P, C2], f32, tag="po",
                                      name=f"po_{sb}_{j}")
                    for mm in range(NMM):
                        nc.tensor.matmul(
                            po,
                            lhsT=pt_tiles[mm // 2][:, mm % 2,
                                                   j * P:(j + 1) * P],
                            rhs=vT_sb[mm],
                            start=(mm == 0), stop=(mm == NMM - 1))
                    rc = normp.tile([P, 1], f32, tag="rc")
                    nc.vector.reciprocal(rc, po[:, C:C + 1])
                    ot = outp.tile([P, C], f16, tag="ot")
                    nc.vector.tensor_scalar_mul(ot, po[:, 0:C], rc)
                    n0 = sb * SB + j * P
                    nc.sync.dma_start(out=outT_d[n0:n0 + P, :], in_=ot)

            # ---- prologue: k/v projections hand-interleaved with the first
            # superblock's scores, following the DMA arrival order, so the PE
            # never drains while x2/x1 chunks trickle in ----
            pt0 = []
            for qt in range(4):
                emit_kqproj(qt * 2)
                emit_kqproj(qt * 2 + 1)
                emit_vproj(qt * 8, 8)
                for t in range(qt * 4, qt * 4 + 4):
                    emit_scores(0, t, pt0)
            emit_out(0, pt0)

            for sb in range(1, NSB):
                pt_tiles = []
                for t in range(NMM // 2):
                    emit_scores(sb, t, pt_tiles)
                emit_out(sb, pt_tiles)
    nc.compile()
    return nc


def _get_program():
    if "nc" not in _CACHE:
        _CACHE["nc"] = _build_program()
    return _CACHE["nc"]


def kernel(**inputs) -> np.ndarray:
    # device layout: [quarter, partition, kc, n] with channel c = kc*128 + p
    def arrange(x):
        x = np.asarray(x, np.float16).reshape(B, KC, P, 4, N // 4)
        return np.ascontiguousarray(x.transpose(0, 3, 2, 1, 4))
    x1 = arrange(inputs["x1"])
    x2 = arrange(inputs["x2"])
    # scores = (Wq x1)^T (Wk x2) = x1^T (Wq^T Wk) x2: fold both score
    # projections into one by shipping G = Wk^T Wq as the k-side weight;
    # x1 then feeds the score matmuls raw (saves 32 matmuls/core and one
    # fp32r rounding on the q side).
    G = (np.asarray(inputs["Wk"], np.float64).T
         @ np.asarray(inputs["Wq"], np.float64))
    wkT = np.ascontiguousarray(G.astype(np.float16))
    wvT = np.ascontiguousarray(np.asarray(inputs["Wv"], np.float16).T)

    in_maps = [
        {"x1": x1[b], "x2": x2[b], "wkT": wkT, "wvT": wvT}
        for b in range(B)
    ]
    nc = _get_program()
    res = bass_utils.run_bass_kernel_spmd(nc, in_maps, core_ids=list(range(B)),
                                          trace=TRACE, tmpdir=TRACE_DIR)
    _CACHE["last_results"] = res
    out = np.empty((B, C, N), np.float32)
    for b in range(B):
        out[b] = res.results[b]["outT"].T.astype(np.float32)
    return out.reshape(B, C, H, W)


if __name__ == "__main__":
    nc = _build_program()
    n = sum(len(b.instructions) for b in nc.m.functions[0].blocks)
    print(f"program built ok: {n} instructions")
